# revision 28
# baseline (speedup 1.0000x reference)
"""Trainium2 Bass kernel for nn_LA_283467842715.

Math (per batch b, head h of 16, each head owning 128 contiguous channels):
  means/maxs over (128 group channels x 2x2 patch) -> [B,16,4,4]
  tiny MLP (16->1 conv, relu, 1->16 conv) on means and maxs, fused by a
  2->1 conv, bilinear-upsampled 4x4->8x8, sigmoid -> gate
  out = x * (1 + gate[b, h, y, x])

Implementation outline (per core: 32 batches, 4 chunks of 8 batches; SBUF
tile X [128, 8192] bf16 with partition p = b*16+h, free = c*64 + y*8 + x):

  mean   : the whole w1-weighted mean reduction is LINEAR, so it runs on the
           otherwise-idle TensorE: 16 PSUM-accumulating matmuls
           (lhsT = W1blk/512 [128,8], rhs = strided X views [p, i, x, c16]
           per (dy, c-group)) contract the head weighting, the channel sum
           and the dy sum at once -> MeanP [8, (i,j,dx,c16)=512].
           One DVE reduce (innermost 32) finishes the cells -> [8, 16].
  max    : DVE y-pair tensor_max (bf16 2x) then a contiguous c-halving
           tensor_max tree down to [i, x8], then a tiny dx-pair fold.
  MLP    : hpre_max = smax^T @ W1blk (PE); mean cells transposed into
           partition-aligned rows via a 32x32 PE transpose; two ScalarE
           relus assemble hcatT [32, 8]; qup = hcatT^T @ K2E (wv-weighted
           bilinear upsample); gpre = W2blk^T @ qup; sigmoid(+beta) + 1.
  out    : X *= gate2 broadcast over c (DVE bf16 2x), DMA back.

Program emission is software-pipelined: chunk k's multiply+store is emitted
after chunk k+1's reductions so the DVE stream never stalls on the tiny MLP
latency.  All HBM traffic is bf16 (host converts f32 <-> bf16).
"""

import sys

if "/opt/trn_rl_repo" not in sys.path:
    sys.path.insert(0, "/opt/trn_rl_repo")

import numpy as np

HEAD = 16
B, C, H, W = 256, 2048, 8, 8
NCORES = 8
BPC = B // NCORES          # 32 batches per core
CHUNK_B = 8                # batches per SBUF chunk (8*16 heads = 128 partitions)
NCHUNK = BPC // CHUNK_B    # 4
C16 = C // HEAD            # 128 channels per head group
SPAT = H * W               # 64
FREE = C16 * SPAT          # 8192 elems per partition per chunk

LAST_EXEC_NS = None        # filled when trace=True


def _upsample_matrix():
    """U[8,4]: bilinear 4->8, half-pixel centers (align_corners=False)."""
    U = np.zeros((8, 4), dtype=np.float64)
    for y in range(8):
        src = (y + 0.5) / 2.0 - 0.5
        i0 = int(np.floor(src))
        t = src - i0
        U[y, min(max(i0, 0), 3)] += 1.0 - t
        U[y, min(max(i0 + 1, 0), 3)] += t
    return U


def _pack_params(w1, b1, w2, b2, wv, bv):
    import ml_dtypes

    w1 = np.asarray(w1, np.float64).reshape(HEAD)
    w2 = np.asarray(w2, np.float64).reshape(HEAD)
    b2 = np.asarray(b2, np.float64).reshape(HEAD)
    wv = np.asarray(wv, np.float64).reshape(2)
    bv = float(np.asarray(bv, np.float64))
    b1 = float(np.asarray(b1, np.float64))

    p = np.arange(128)
    h16 = p % HEAD
    blk = p // HEAD  # which batch-slot this partition belongs to

    # fp32 consts [128, 35]: col 0 betacol, col 1 b1, cols 2:34 identity
    # (rows 0:32) for the PE transpose.
    cf32 = np.zeros((128, 35), np.float64)
    cf32[:, 0] = (wv[0] + wv[1]) * b2[h16] + bv
    cf32[:, 1] = b1
    cf32[0:32, 2:34] = np.eye(32)

    U = _upsample_matrix()
    # K2[g, s] = U[y,i] * U[x,j], g = i*4+j, s = y*8+x
    K2 = np.einsum("yi,xj->ijyx", U, U).reshape(16, 64)

    # bf16 consts [128, 208]:
    #   rows 0:32,  cols 0:64    K2E: wv1*K2 for the max half (hcat rows
    #                            0:16), wv0*K2 for the mean half (16:32)
    #   rows 0:128, cols 64:72   W1m[p, b] = (p//16==b) * w1[p%16] / 512
    #   rows 0:128, cols 72:80   W1x[p, b] = (p//16==b) * w1[p%16]
    #   rows 0:8,   cols 80:208  W2blk[b, p] = (p//16==b) * w2[p%16]
    cbf = np.zeros((128, 208), np.float64)
    cbf[0:16, 0:64] = wv[1] * K2
    cbf[16:32, 0:64] = wv[0] * K2
    for b in range(CHUNK_B):
        cbf[:, 64 + b] = np.where(blk == b, w1[h16] / 512.0, 0.0)
        cbf[:, 72 + b] = np.where(blk == b, w1[h16], 0.0)
        cbf[b, 80:208] = np.where(blk == b, w2[h16], 0.0)

    return {
        "cf32": np.ascontiguousarray(cf32, np.float32),
        "cbf": np.ascontiguousarray(cbf.astype(np.float32), ml_dtypes.bfloat16),
    }


def _chain_input_dmas(nc, mybir, window=6):
    """Sliding-window chaining of the x-input quarter-DMAs: quarter i waits
    for quarter i-window.  Unchained, all 16 quarters flood the queues
    round-robin and chunk 0 — which gates all compute — arrives ~4x late;
    fully serialized, too few queues are active to reach full HBM bandwidth
    (one DMA queue sustains only ~100 GB/s).  A window of ~6 keeps ~3 MB in
    flight: saturated HBM and in-order arrival."""
    cum = {}
    xdmas = []
    for fn in nc.m.functions:
        for bb in fn.blocks:
            for ins in bb.instructions:
                si = getattr(ins, "sync_info", None)
                if si is None:
                    continue
                ups = list(si.on_update) if si.on_update else []
                for u in ups:
                    cum[u.id] = cum.get(u.id, 0) + u.update_value
                if (type(ins).__name__ == "InstDMACopy"
                        and str(getattr(ins.ins[0], "memref", "")) == "x"):
                    u = ups[0]
                    xdmas.append((ins, (u.id, u.ant_name, cum[u.id]),
                                  ins.ins[0].offset))
    xdmas.sort(key=lambda t: t[2])
    for i in range(4, len(xdmas)):
        win = 4 if i < 8 else window
        if i < win:
            continue
        sem_id, name, val = xdmas[i - win][1]
        ins = xdmas[i][0]
        w = mybir.SyncWait(sync_type="semaphore", id=sem_id,
                          wait_mode="sem-ge-imm", wait_value=val,
                          ant_name=name)
        ins.sync_info.on_wait = list(ins.sync_info.on_wait or []) + [w]


def _split_multi_waits(nc, mybir):
    """Walrus codegen supports one sync-wait per instruction; hoist extras
    onto standalone InstEventSemaphore waits inserted right before, on the
    same engine (engines execute their stream in order, so this preserves
    the happens-before edges)."""
    n = 0
    for fn in nc.m.functions:
        for bb in fn.blocks:
            out = []
            for ins in bb.instructions:
                si = getattr(ins, "sync_info", None)
                waits = list(si.on_wait) if (si and si.on_wait) else []
                if len(waits) > 1:
                    for w in waits[:-1]:
                        n += 1
                        ev = mybir.InstEventSemaphore(
                            name=f"WSPLIT-{n}",
                            sync_info=mybir.SyncInfo(on_wait=[w], on_update=[]),
                        )
                        ev.engine = ins.engine
                        out.append(ev)
                    si.on_wait = [waits[-1]]
                out.append(ins)
            bb.instructions[:] = out


def _build(split_waits=True):
    import concourse.bass as bass
    import concourse.tile as tile
    from concourse import mybir

    f32 = mybir.dt.float32
    bf16 = mybir.dt.bfloat16
    nc = bass.Bass()

    xd = nc.dram_tensor("x", [NCHUNK, 128, FREE], bf16, kind="ExternalInput")
    od = nc.dram_tensor("out", [NCHUNK, 128, FREE], bf16, kind="ExternalOutput")
    cf32d = nc.dram_tensor("cf32", [128, 35], f32, kind="ExternalInput")
    cbfd = nc.dram_tensor("cbf", [128, 208], bf16, kind="ExternalInput")

    AF = mybir.ActivationFunctionType

    with tile.TileContext(nc) as tc:
        with (
            tc.tile_pool(name="singles", bufs=1) as singles,
            tc.tile_pool(name="xin", bufs=4) as xpool,
            tc.tile_pool(name="mid", bufs=2) as mid,
            tc.tile_pool(name="small", bufs=3) as small,
            tc.tile_pool(name="pmean", bufs=2, space="PSUM") as pmean,
            tc.tile_pool(name="pgpre", bufs=2, space="PSUM") as pgpre,
            tc.tile_pool(name="psmall", bufs=1, space="PSUM") as psmall,
        ):
            # Tiny const DMAs first (the scheduler plans around their
            # completion; issuing them after the bulk input DMAs makes it
            # believe the weights arrive late and it defers the whole PE
            # pipeline).  Then the input DMAs for every chunk: 4 quarter
            # DMAs per chunk on the SP sequencer; the post-pass chains them
            # with a sliding window.
            s_cf32 = singles.tile([128, 35], f32)
            nc.sync.dma_start(out=s_cf32, in_=cf32d[:, :])
            s_cbf = singles.tile([128, 208], bf16)
            nc.sync.dma_start(out=s_cbf, in_=cbfd[:, :])

            # 8 eighth-DMAs per chunk: ~8 concurrent transfers keep enough
            # descriptors in flight to hide HBM latency on all 16 DMA
            # engines.  Chunks 0-1 issue from the (otherwise idle early on)
            # Activation sequencer, which starts ~4us before SP gets through
            # its preamble; chunks 2-3 from SP.
            # tile_wait_until paces the quarters in the SCHEDULER'S sim to
            # match their real (chained) arrival order — without it the sim
            # assumes all 16 quarters flood round-robin, concludes late
            # chunks' data arrives very late, and statically orders their
            # reductions after older chunks' multiplies, which in reality
            # strands the critical path.
            QRT = FREE // 4
            xtiles = []
            for ci in range(NCHUNK):
                X = xpool.tile([128, FREE], bf16, tag="X")
                for q in range(4):
                    nc.sync.dma_start(out=X[:, q * QRT:(q + 1) * QRT],
                                      in_=xd[ci, :, q * QRT:(q + 1) * QRT])
                xtiles.append(X)
            s_beta = s_cf32[:, 0:1]
            s_b1 = s_cf32[:, 1:2]
            s_ident = s_cf32[0:32, 2:34]
            s_k2e = s_cbf[0:32, 0:64]
            s_w1m = s_cbf[:, 64:72]
            s_w1x = s_cbf[:, 72:80]
            s_w2blk = s_cbf[0:8, 80:208]

            # Absorb the const DMAs' semaphores into engine vector clocks
            # (walrus supports one sync-wait per instruction) and pre-load
            # the sigmoid ACT table set during the input-DMA ramp.
            d_a = singles.tile([1, 1], f32, tag="d_a")
            nc.scalar.activation(d_a, s_cf32[0:1, 0:1], AF.Sigmoid)
            d_v = singles.tile([1, 2], bf16, tag="d_v")
            nc.vector.tensor_copy(d_v, s_cbf[0:1, 0:2])
            # GpSimd warmup: absorb the one-time Q7 dispatch/IRAM cost before
            # the multiply offload needs it.
            d_g = singles.tile([128, 2], bf16, tag="d_g")
            nc.gpsimd.tensor_mul(d_g, s_cbf[:, 0:2], s_cbf[:, 0:2])

            # [32,32] staging tile for the mean cells: DVE writes the cell
            # sums into cols 16:32 rows 0:8; a PE transpose then lands them
            # partition-aligned at rows 16:32.  memset once so the PE
            # transpose never reads uninitialized SBUF.
            mean8pad = singles.tile([32, 32], f32, tag="mean8pad")
            nc.vector.memset(mean8pad[:, :], 0.0)

            def emit_reduce(ci):
                """mean (PE) + max (DVE) + MLP -> gate2 for chunk ci.
                Returns (X, gate2) for the deferred multiply/store."""
                X = xtiles[ci]
                Xf = X[:, :]

                # Mean path on TensorE: 16 matmuls over fully CONTIGUOUS
                # 512-column slices of X (strided rhs APs stall the PE
                # stream ~4x).  Since the mean sums over all channels, every
                # slice accumulates into the same PSUM columns (c%8, y, x):
                # MeanP[b, (c8,y,x)] = sum_p W1m[p,b] * sum_slices X.
                meanp = pmean.tile([8, 512], f32, tag="meanp")
                for k in range(16):
                    nc.tensor.matmul(meanp[:, :], s_w1m,
                                     Xf[:, k * 512:(k + 1) * 512],
                                     start=(k == 0), stop=(k == 15))

                # Max path: y-pair fold (two c-halves so each starts as its
                # input quarters land), then a contiguous halving tree, and
                # one strided reduce for the (c2, dx) tail.
                X5 = Xf.rearrange("p (c i dy x) -> p c i dy x",
                                  c=C16, i=4, dy=2, x=8)
                tm = mid.tile([128, 4096], bf16, tag="tm")
                tmv = tm[:, :].rearrange("p (c i x) -> p c i x", c=C16, i=4, x=8)
                # chunk 0's input quarters arrive staggered, so fold per
                # quarter there; later chunks' data is resident by the time
                # the DVE reaches them, so fewer (cheaper) ops win.
                nfold = 4 if ci == 0 else 2
                for ch in range(nfold):
                    cl = slice(ch * (C16 // nfold), (ch + 1) * (C16 // nfold))
                    nc.vector.tensor_max(
                        tmv[:, cl], X5[:, cl, :, 0, :], X5[:, cl, :, 1, :])
                n = 4096
                while n > 64:
                    n //= 2
                    nc.vector.tensor_max(tm[:, 0:n], tm[:, 0:n], tm[:, n:2 * n])
                smax = small.tile([128, 16], bf16, tag="smax")
                tmd = tm[:, 0:64].rearrange("p (c i j dx) -> p i j c dx",
                                            c=2, i=4, j=4, dx=2)
                nc.vector.reduce_max(out=smax[:, :], in_=tmd,
                                     axis=mybir.AxisListType.XY)

                # mean cells: ScalarE copies the PSUM partials to SBUF (DVE
                # can read only one PSUM operand per op), DVE folds the c8
                # halves at 2x and reduces the 2x2 patch, then a PE
                # transpose lands the cells on partitions 16:32.
                meanpS = small.tile([8, 512], bf16, tag="meanpS")
                nc.scalar.copy(meanpS[:, :], meanp[:, :])
                for n in (256, 128, 64):
                    nc.vector.tensor_add(meanpS[:, 0:n], meanpS[:, 0:n],
                                         meanpS[:, n:2 * n])
                mpv = meanpS[:, 0:64].rearrange("p (i dy j dx) -> p i j dy dx",
                                                i=4, dy=2, j=4, dx=2)
                nc.vector.reduce_sum(out=mean8pad[0:8, 16:32], in_=mpv,
                                     axis=mybir.AxisListType.XY)
                meant = psmall.tile([32, 32], f32, tag="meant")
                nc.tensor.transpose(meant[:, :], mean8pad[:, :], s_ident)
                # hpre_max[cell, b] overwrites rows 0:16 of the same PSUM
                # tile (PSUM reads must start at partition 0, so both halves
                # must live in one offset-0 tile for a single relu).
                nc.tensor.matmul(meant[0:16, 0:8], smax[:, :], s_w1x)

                # hcatT [32, 8]: rows 0:16 = relu(hpre_max + b1),
                # rows 16:32 = relu(hpre_mean + b1)
                hcat = small.tile([32, 8], bf16, tag="hcat")
                nc.scalar.activation(hcat[:, :], meant[0:32, 0:8], AF.Relu,
                                     bias=s_b1[0:32, :])

                qup = psmall.tile([8, 64], f32, tag="qup")
                nc.tensor.matmul(qup[:, :], hcat[:, :], s_k2e)
                qupS = small.tile([8, 64], bf16, tag="qupS")
                nc.scalar.copy(qupS[:, :], qup[:, :])

                gpre = pgpre.tile([128, 64], f32, tag="gpre")
                nc.tensor.matmul(gpre[:, :], s_w2blk, qupS[:, :])

                gate = small.tile([128, 64], bf16, tag="gate")
                nc.scalar.activation(gate[:, :], gpre[:, :], AF.Sigmoid,
                                     bias=s_beta)
                gate2 = small.tile([128, 64], bf16, tag="gate2")
                nc.scalar.add(gate2[:, :], gate[:, :], 1.0)
                return X, gate2

            def emit_mul_store(ci, X, gate2):
                """out = gate2 * x (gate2 broadcast over the 128 group
                channels), store.  Pieces let the store overlap the mul.
                Output DMAs issue on the SP sequencer: by the time the first
                store is ready all chained input DMAs have issued, while the
                Activation sequencer must stay free for the MLP chain (a
                pending out-DMA's wait would stall the next chunk's
                relu/sigmoid and delay its gate).  The last chunk's stores
                are split into two parallel streams each so the final drain
                is not single-stream limited."""
                # The otherwise-idle GpSimd engine multiplies the first 40
                # channels (~5.5us, concurrent with the DVE pieces; GpSimd's
                # SBUF port only contends with DVE 2-PORT modes, which this
                # kernel never uses).  Not on the last chunk, where GpSimd's
                # slower rate would extend the kernel tail.
                pieces = []
                if ci < NCHUNK - 1:
                    pieces.append((slice(0, 40), nc.gpsimd))
                    for cl in ((40, 72), (72, 100), (100, 128)):
                        pieces.append((slice(*cl), nc.vector))
                else:
                    for cl in ((0, 32), (32, 64), (64, 96), (96, 128)):
                        pieces.append((slice(*cl), nc.vector))
                nstream = 2 if ci == NCHUNK - 1 else 1
                for csl, eng in pieces:
                    cs = csl.stop - csl.start
                    sl = slice(csl.start * SPAT, csl.stop * SPAT)
                    g_bc = gate2[:, :].unsqueeze(1).broadcast_to([128, cs, SPAT])
                    X3 = X[:, sl].rearrange("p (c s) -> p c s", s=SPAT)
                    eng.tensor_mul(X3, g_bc, X3)
                    step = cs * SPAT // nstream
                    for st in range(nstream):
                        ssl = slice(sl.start + st * step,
                                    sl.start + (st + 1) * step)
                        nc.sync.dma_start(out=od[ci, :, ssl], in_=X[:, ssl])

            pend = None
            for ci in range(NCHUNK):
                cur = emit_reduce(ci)
                if pend is not None:
                    emit_mul_store(ci - 1, *pend)
                pend = cur
            emit_mul_store(NCHUNK - 1, *pend)

    if split_waits:
        _split_multi_waits(nc, mybir)
    return nc


def _shard_inputs(x, consts):
    import ml_dtypes

    xb = np.ascontiguousarray(x).astype(ml_dtypes.bfloat16)
    in_maps = []
    for i in range(NCORES):
        shard = xb[i * BPC:(i + 1) * BPC]  # [32, 2048, 8, 8]
        m = {"x": np.ascontiguousarray(shard.reshape(NCHUNK, 128, FREE))}
        m.update(consts)
        in_maps.append(m)
    return in_maps


def kernel(x, w1, b1, w2, b2, wv, bv, trace=False):
    global LAST_EXEC_NS
    from concourse.bass_utils import run_bass_kernel_spmd

    x = np.asarray(x, np.float32)
    consts = _pack_params(w1, b1, w2, b2, wv, bv)
    nc = _build()
    in_maps = _shard_inputs(x, consts)

    res = run_bass_kernel_spmd(nc, in_maps, core_ids=list(range(NCORES)),
                               trace=trace)
    LAST_EXEC_NS = res.exec_time_ns

    out = np.empty((B, C, H, W), np.float32)
    for i, r in enumerate(res.results):
        out[i * BPC:(i + 1) * BPC] = np.asarray(r["out"], np.float32).reshape(
            BPC, C, H, W)
    return out


# revision 30
# speedup vs baseline: 1.1337x; 1.1337x over previous
"""Trainium2 Bass kernel for nn_LA_283467842715.

Math (per batch b, head h of 16, each head owning 128 contiguous channels):
  means/maxs over (128 group channels x 2x2 patch) -> [B,16,4,4]
  tiny MLP (16->1 conv, relu, 1->16 conv) on means and maxs, fused by a
  2->1 conv, bilinear-upsampled 4x4->8x8, sigmoid -> gate
  out = x * (1 + gate[b, h, y, x])

Implementation outline (per core: 32 batches, 4 chunks of 8 batches; SBUF
tile X [128, 8192] bf16 with partition p = b*16+h, free = c*64 + y*8 + x):

  mean   : the whole w1-weighted mean reduction is LINEAR, so it runs on the
           otherwise-idle TensorE: 16 PSUM-accumulating matmuls
           (lhsT = W1blk/512 [128,8], rhs = strided X views [p, i, x, c16]
           per (dy, c-group)) contract the head weighting, the channel sum
           and the dy sum at once -> MeanP [8, (i,j,dx,c16)=512].
           One DVE reduce (innermost 32) finishes the cells -> [8, 16].
  max    : DVE y-pair tensor_max (bf16 2x) then a contiguous c-halving
           tensor_max tree down to [i, x8], then a tiny dx-pair fold.
  MLP    : hpre_max = smax^T @ W1blk (PE); mean cells transposed into
           partition-aligned rows via a 32x32 PE transpose; two ScalarE
           relus assemble hcatT [32, 8]; qup = hcatT^T @ K2E (wv-weighted
           bilinear upsample); gpre = W2blk^T @ qup; sigmoid(+beta) + 1.
  out    : X *= gate2 broadcast over c (DVE bf16 2x), DMA back.

Program emission is software-pipelined: chunk k's multiply+store is emitted
after chunk k+1's reductions so the DVE stream never stalls on the tiny MLP
latency.  All HBM traffic is bf16 (host converts f32 <-> bf16).
"""

import sys

if "/opt/trn_rl_repo" not in sys.path:
    sys.path.insert(0, "/opt/trn_rl_repo")

import numpy as np

HEAD = 16
B, C, H, W = 256, 2048, 8, 8
NCORES = 8
BPC = B // NCORES          # 32 batches per core
CHUNK_B = 8                # batches per SBUF chunk (8*16 heads = 128 partitions)
NCHUNK = BPC // CHUNK_B    # 4
C16 = C // HEAD            # 128 channels per head group
SPAT = H * W               # 64
FREE = C16 * SPAT          # 8192 elems per partition per chunk

LAST_EXEC_NS = None        # filled when trace=True


def _upsample_matrix():
    """U[8,4]: bilinear 4->8, half-pixel centers (align_corners=False)."""
    U = np.zeros((8, 4), dtype=np.float64)
    for y in range(8):
        src = (y + 0.5) / 2.0 - 0.5
        i0 = int(np.floor(src))
        t = src - i0
        U[y, min(max(i0, 0), 3)] += 1.0 - t
        U[y, min(max(i0 + 1, 0), 3)] += t
    return U


def _pack_params(w1, b1, w2, b2, wv, bv):
    import ml_dtypes

    w1 = np.asarray(w1, np.float64).reshape(HEAD)
    w2 = np.asarray(w2, np.float64).reshape(HEAD)
    b2 = np.asarray(b2, np.float64).reshape(HEAD)
    wv = np.asarray(wv, np.float64).reshape(2)
    bv = float(np.asarray(bv, np.float64))
    b1 = float(np.asarray(b1, np.float64))

    p = np.arange(128)
    h16 = p % HEAD
    blk = p // HEAD  # which batch-slot this partition belongs to

    # fp32 consts [128, 35]: col 0 betacol, col 1 b1, cols 2:34 identity
    # (rows 0:32) for the PE transpose.
    cf32 = np.zeros((128, 35), np.float64)
    cf32[:, 0] = (wv[0] + wv[1]) * b2[h16] + bv
    cf32[:, 1] = b1
    cf32[0:32, 2:34] = np.eye(32)

    U = _upsample_matrix()
    # K2[g, s] = U[y,i] * U[x,j], g = i*4+j, s = y*8+x
    K2 = np.einsum("yi,xj->ijyx", U, U).reshape(16, 64)

    # bf16 consts [128, 208]:
    #   rows 0:32,  cols 0:64    K2E: wv1*K2 for the max half (hcat rows
    #                            0:16), wv0*K2 for the mean half (16:32)
    #   rows 0:128, cols 64:72   W1m[p, b] = (p//16==b) * w1[p%16] / 512
    #   rows 0:128, cols 72:80   W1x[p, b] = (p//16==b) * w1[p%16]
    #   rows 0:8,   cols 80:208  W2blk[b, p] = (p//16==b) * w2[p%16]
    cbf = np.zeros((128, 208), np.float64)
    cbf[0:16, 0:64] = wv[1] * K2
    cbf[16:32, 0:64] = wv[0] * K2
    for b in range(CHUNK_B):
        cbf[:, 64 + b] = np.where(blk == b, w1[h16] / 512.0, 0.0)
        cbf[:, 72 + b] = np.where(blk == b, w1[h16], 0.0)
        cbf[b, 80:208] = np.where(blk == b, w2[h16], 0.0)

    return {
        "cf32": np.ascontiguousarray(cf32, np.float32),
        "cbf": np.ascontiguousarray(cbf.astype(np.float32), ml_dtypes.bfloat16),
    }


def _chain_input_dmas(nc, mybir, window=6):
    """Sliding-window chaining of the x-input quarter-DMAs: quarter i waits
    for quarter i-window.  Unchained, all 16 quarters flood the queues
    round-robin and chunk 0 — which gates all compute — arrives ~4x late;
    fully serialized, too few queues are active to reach full HBM bandwidth
    (one DMA queue sustains only ~100 GB/s).  A window of ~6 keeps ~3 MB in
    flight: saturated HBM and in-order arrival."""
    cum = {}
    xdmas = []
    for fn in nc.m.functions:
        for bb in fn.blocks:
            for ins in bb.instructions:
                si = getattr(ins, "sync_info", None)
                if si is None:
                    continue
                ups = list(si.on_update) if si.on_update else []
                for u in ups:
                    cum[u.id] = cum.get(u.id, 0) + u.update_value
                if (type(ins).__name__ == "InstDMACopy"
                        and str(getattr(ins.ins[0], "memref", "")) == "x"):
                    u = ups[0]
                    xdmas.append((ins, (u.id, u.ant_name, cum[u.id]),
                                  ins.ins[0].offset))
    xdmas.sort(key=lambda t: t[2])
    for i in range(4, len(xdmas)):
        win = 4 if i < 8 else window
        if i < win:
            continue
        sem_id, name, val = xdmas[i - win][1]
        ins = xdmas[i][0]
        w = mybir.SyncWait(sync_type="semaphore", id=sem_id,
                          wait_mode="sem-ge-imm", wait_value=val,
                          ant_name=name)
        ins.sync_info.on_wait = list(ins.sync_info.on_wait or []) + [w]


def _split_multi_waits(nc, mybir):
    """Walrus codegen supports one sync-wait per instruction; hoist extras
    onto standalone InstEventSemaphore waits inserted right before, on the
    same engine (engines execute their stream in order, so this preserves
    the happens-before edges)."""
    n = 0
    for fn in nc.m.functions:
        for bb in fn.blocks:
            out = []
            for ins in bb.instructions:
                si = getattr(ins, "sync_info", None)
                waits = list(si.on_wait) if (si and si.on_wait) else []
                if len(waits) > 1:
                    for w in waits[:-1]:
                        n += 1
                        ev = mybir.InstEventSemaphore(
                            name=f"WSPLIT-{n}",
                            sync_info=mybir.SyncInfo(on_wait=[w], on_update=[]),
                        )
                        ev.engine = ins.engine
                        out.append(ev)
                    si.on_wait = [waits[-1]]
                out.append(ins)
            bb.instructions[:] = out


def _build(split_waits=True):
    import concourse.bass as bass
    import concourse.tile as tile
    from concourse import mybir

    f32 = mybir.dt.float32
    bf16 = mybir.dt.bfloat16
    nc = bass.Bass()

    xd = nc.dram_tensor("x", [NCHUNK, 128, FREE], bf16, kind="ExternalInput")
    od = nc.dram_tensor("out", [NCHUNK, 128, FREE], bf16, kind="ExternalOutput")
    cf32d = nc.dram_tensor("cf32", [128, 35], f32, kind="ExternalInput")
    cbfd = nc.dram_tensor("cbf", [128, 208], bf16, kind="ExternalInput")

    AF = mybir.ActivationFunctionType

    with tile.TileContext(nc) as tc:
        with (
            tc.tile_pool(name="singles", bufs=1) as singles,
            tc.tile_pool(name="xin", bufs=4) as xpool,
            tc.tile_pool(name="mid", bufs=2) as mid,
            tc.tile_pool(name="small", bufs=3) as small,
            tc.tile_pool(name="pmean", bufs=2, space="PSUM") as pmean,
            tc.tile_pool(name="pgpre", bufs=2, space="PSUM") as pgpre,
            tc.tile_pool(name="psmall", bufs=1, space="PSUM") as psmall,
        ):
            # Tiny const DMAs first (the scheduler plans around their
            # completion; issuing them after the bulk input DMAs makes it
            # believe the weights arrive late and it defers the whole PE
            # pipeline).  Then the input DMAs for every chunk: 4 quarter
            # DMAs per chunk on the SP sequencer; the post-pass chains them
            # with a sliding window.
            s_cf32 = singles.tile([128, 35], f32)
            nc.sync.dma_start(out=s_cf32, in_=cf32d[:, :])
            s_cbf = singles.tile([128, 208], bf16)
            nc.sync.dma_start(out=s_cbf, in_=cbfd[:, :])

            # 8 eighth-DMAs per chunk: ~8 concurrent transfers keep enough
            # descriptors in flight to hide HBM latency on all 16 DMA
            # engines.  Chunks 0-1 issue from the (otherwise idle early on)
            # Activation sequencer, which starts ~4us before SP gets through
            # its preamble; chunks 2-3 from SP.
            # tile_wait_until paces the quarters in the SCHEDULER'S sim to
            # match their real (chained) arrival order — without it the sim
            # assumes all 16 quarters flood round-robin, concludes late
            # chunks' data arrives very late, and statically orders their
            # reductions after older chunks' multiplies, which in reality
            # strands the critical path.
            QRT = FREE // 4
            xtiles = []
            for ci in range(NCHUNK):
                X = xpool.tile([128, FREE], bf16, tag="X")
                for q in range(4):
                    nc.sync.dma_start(out=X[:, q * QRT:(q + 1) * QRT],
                                      in_=xd[ci, :, q * QRT:(q + 1) * QRT])
                xtiles.append(X)
            s_beta = s_cf32[:, 0:1]
            s_b1 = s_cf32[:, 1:2]
            s_ident = s_cf32[0:32, 2:34]
            s_k2e = s_cbf[0:32, 0:64]
            s_w1m = s_cbf[:, 64:72]
            s_w1x = s_cbf[:, 72:80]
            s_w2blk = s_cbf[0:8, 80:208]

            # Absorb the const DMAs' semaphores into engine vector clocks
            # (walrus supports one sync-wait per instruction) and pre-load
            # the sigmoid ACT table set during the input-DMA ramp.
            d_a = singles.tile([1, 1], f32, tag="d_a")
            nc.scalar.activation(d_a, s_cf32[0:1, 0:1], AF.Sigmoid)
            d_v = singles.tile([1, 2], bf16, tag="d_v")
            nc.vector.tensor_copy(d_v, s_cbf[0:1, 0:2])
            # GpSimd warmup: absorb the one-time Q7 dispatch/IRAM cost before
            # the multiply offload needs it.
            d_g = singles.tile([128, 2], bf16, tag="d_g")
            nc.gpsimd.tensor_mul(d_g, s_cbf[:, 0:2], s_cbf[:, 0:2])

            # [32,32] staging tile for the mean cells: DVE writes the cell
            # sums into cols 16:32 rows 0:8; a PE transpose then lands them
            # partition-aligned at rows 16:32.  memset once so the PE
            # transpose never reads uninitialized SBUF.
            mean8pad = singles.tile([32, 32], f32, tag="mean8pad")
            nc.vector.memset(mean8pad[:, :], 0.0)

            def emit_reduce(ci):
                """mean (PE) + max (DVE) + MLP -> gate2 for chunk ci.
                Returns (X, gate2) for the deferred multiply/store."""
                X = xtiles[ci]
                Xf = X[:, :]

                # Mean path on TensorE: 16 matmuls over fully CONTIGUOUS
                # 512-column slices of X (strided rhs APs stall the PE
                # stream ~4x).  Since the mean sums over all channels, every
                # slice accumulates into the same PSUM columns (c%8, y, x):
                # MeanP[b, (c8,y,x)] = sum_p W1m[p,b] * sum_slices X.
                meanp = pmean.tile([8, 512], f32, tag="meanp")
                for k in range(16):
                    nc.tensor.matmul(meanp[:, :], s_w1m,
                                     Xf[:, k * 512:(k + 1) * 512],
                                     start=(k == 0), stop=(k == 15))

                # Max path: y-pair fold (two c-halves so each starts as its
                # input quarters land), then a contiguous halving tree, and
                # one strided reduce for the (c2, dx) tail.
                X5 = Xf.rearrange("p (c i dy x) -> p c i dy x",
                                  c=C16, i=4, dy=2, x=8)
                tm = mid.tile([128, 4096], bf16, tag="tm")
                tmv = tm[:, :].rearrange("p (c i x) -> p c i x", c=C16, i=4, x=8)
                # chunk 0's input quarters arrive staggered, so fold per
                # quarter there; later chunks' data is resident by the time
                # the DVE reaches them, so fewer (cheaper) ops win.
                nfold = 4 if ci == 0 else 2
                for ch in range(nfold):
                    cl = slice(ch * (C16 // nfold), (ch + 1) * (C16 // nfold))
                    nc.vector.tensor_max(
                        tmv[:, cl], X5[:, cl, :, 0, :], X5[:, cl, :, 1, :])
                n = 4096
                while n > 64:
                    n //= 2
                    nc.vector.tensor_max(tm[:, 0:n], tm[:, 0:n], tm[:, n:2 * n])
                smax = small.tile([128, 16], bf16, tag="smax")
                tmd = tm[:, 0:64].rearrange("p (c i j dx) -> p i j c dx",
                                            c=2, i=4, j=4, dx=2)
                nc.vector.reduce_max(out=smax[:, :], in_=tmd,
                                     axis=mybir.AxisListType.XY)

                # mean cells: ScalarE copies the PSUM partials to SBUF (DVE
                # can read only one PSUM operand per op), DVE folds the c8
                # halves at 2x and reduces the 2x2 patch, then a PE
                # transpose lands the cells on partitions 16:32.
                meanpS = small.tile([8, 512], bf16, tag="meanpS")
                nc.scalar.copy(meanpS[:, :], meanp[:, :])
                for n in (256, 128, 64):
                    nc.vector.tensor_add(meanpS[:, 0:n], meanpS[:, 0:n],
                                         meanpS[:, n:2 * n])
                mpv = meanpS[:, 0:64].rearrange("p (i dy j dx) -> p i j dy dx",
                                                i=4, dy=2, j=4, dx=2)
                nc.vector.reduce_sum(out=mean8pad[0:8, 16:32], in_=mpv,
                                     axis=mybir.AxisListType.XY)
                meant = psmall.tile([32, 32], f32, tag="meant")
                nc.tensor.transpose(meant[:, :], mean8pad[:, :], s_ident)
                # hpre_max[cell, b] overwrites rows 0:16 of the same PSUM
                # tile (PSUM reads must start at partition 0, so both halves
                # must live in one offset-0 tile for a single relu).
                nc.tensor.matmul(meant[0:16, 0:8], smax[:, :], s_w1x)

                # hcatT [32, 8]: rows 0:16 = relu(hpre_max + b1),
                # rows 16:32 = relu(hpre_mean + b1)
                hcat = small.tile([32, 8], bf16, tag="hcat")
                nc.scalar.activation(hcat[:, :], meant[0:32, 0:8], AF.Relu,
                                     bias=s_b1[0:32, :])

                qup = psmall.tile([8, 64], f32, tag="qup")
                nc.tensor.matmul(qup[:, :], hcat[:, :], s_k2e)
                qupS = small.tile([8, 64], bf16, tag="qupS")
                nc.scalar.copy(qupS[:, :], qup[:, :])

                gpre = pgpre.tile([128, 64], f32, tag="gpre")
                nc.tensor.matmul(gpre[:, :], s_w2blk, qupS[:, :])

                gate = small.tile([128, 64], bf16, tag="gate")
                nc.scalar.activation(gate[:, :], gpre[:, :], AF.Sigmoid,
                                     bias=s_beta)
                gate2 = small.tile([128, 64], bf16, tag="gate2")
                nc.scalar.add(gate2[:, :], gate[:, :], 1.0)
                return X, gate2

            def emit_mul_store(ci, X, gate2):
                """out = gate2 * x (gate2 broadcast over the 128 group
                channels), store.  Pieces let the store overlap the mul.
                Output DMAs issue on the SP sequencer: by the time the first
                store is ready all chained input DMAs have issued, while the
                Activation sequencer must stay free for the MLP chain (a
                pending out-DMA's wait would stall the next chunk's
                relu/sigmoid and delay its gate).  The last chunk's stores
                are split into two parallel streams each so the final drain
                is not single-stream limited."""
                nstream = 2 if ci == NCHUNK - 1 else 1
                for cl in ((0, 32), (32, 64), (64, 96), (96, 128)):
                    cs = cl[1] - cl[0]
                    sl = slice(cl[0] * SPAT, cl[1] * SPAT)
                    g_bc = gate2[:, :].unsqueeze(1).broadcast_to([128, cs, SPAT])
                    X3 = X[:, sl].rearrange("p (c s) -> p c s", s=SPAT)
                    nc.vector.tensor_mul(X3, g_bc, X3)
                    step = cs * SPAT // nstream
                    for st in range(nstream):
                        ssl = slice(sl.start + st * step,
                                    sl.start + (st + 1) * step)
                        nc.sync.dma_start(out=od[ci, :, ssl], in_=X[:, ssl])

            # Defer each chunk's multiply by TWO chunks: every gate is then
            # ~6us-deep ready when its multiply's turn comes, so neither the
            # kernel tail nor scheduler reorderings ever wait on the tiny
            # MLP chain's latency.
            LAG = 2
            pend = []
            for ci in range(NCHUNK):
                pend.append(emit_reduce(ci))
                if ci >= LAG:
                    emit_mul_store(ci - LAG, *pend[ci - LAG])
            for ci in range(NCHUNK - LAG, NCHUNK):
                emit_mul_store(ci, *pend[ci])

    if split_waits:
        _split_multi_waits(nc, mybir)
    return nc


def _shard_inputs(x, consts):
    import ml_dtypes

    xb = np.ascontiguousarray(x).astype(ml_dtypes.bfloat16)
    in_maps = []
    for i in range(NCORES):
        shard = xb[i * BPC:(i + 1) * BPC]  # [32, 2048, 8, 8]
        m = {"x": np.ascontiguousarray(shard.reshape(NCHUNK, 128, FREE))}
        m.update(consts)
        in_maps.append(m)
    return in_maps


def kernel(x, w1, b1, w2, b2, wv, bv, trace=False):
    global LAST_EXEC_NS
    from concourse.bass_utils import run_bass_kernel_spmd

    x = np.asarray(x, np.float32)
    consts = _pack_params(w1, b1, w2, b2, wv, bv)
    nc = _build()
    in_maps = _shard_inputs(x, consts)

    res = run_bass_kernel_spmd(nc, in_maps, core_ids=list(range(NCORES)),
                               trace=trace)
    LAST_EXEC_NS = res.exec_time_ns

    out = np.empty((B, C, H, W), np.float32)
    for i, r in enumerate(res.results):
        out[i * BPC:(i + 1) * BPC] = np.asarray(r["out"], np.float32).reshape(
            BPC, C, H, W)
    return out


# revision 31
# speedup vs baseline: 1.1435x; 1.0087x over previous
"""Trainium2 Bass kernel for nn_LA_283467842715.

Math (per batch b, head h of 16, each head owning 128 contiguous channels):
  means/maxs over (128 group channels x 2x2 patch) -> [B,16,4,4]
  tiny MLP (16->1 conv, relu, 1->16 conv) on means and maxs, fused by a
  2->1 conv, bilinear-upsampled 4x4->8x8, sigmoid -> gate
  out = x * (1 + gate[b, h, y, x])

Implementation outline (per core: 32 batches, 4 chunks of 8 batches; SBUF
tile X [128, 8192] bf16 with partition p = b*16+h, free = c*64 + y*8 + x):

  mean   : the whole w1-weighted mean reduction is LINEAR, so it runs on the
           otherwise-idle TensorE: 16 PSUM-accumulating matmuls
           (lhsT = W1blk/512 [128,8], rhs = strided X views [p, i, x, c16]
           per (dy, c-group)) contract the head weighting, the channel sum
           and the dy sum at once -> MeanP [8, (i,j,dx,c16)=512].
           One DVE reduce (innermost 32) finishes the cells -> [8, 16].
  max    : DVE y-pair tensor_max (bf16 2x) then a contiguous c-halving
           tensor_max tree down to [i, x8], then a tiny dx-pair fold.
  MLP    : hpre_max = smax^T @ W1blk (PE); mean cells transposed into
           partition-aligned rows via a 32x32 PE transpose; two ScalarE
           relus assemble hcatT [32, 8]; qup = hcatT^T @ K2E (wv-weighted
           bilinear upsample); gpre = W2blk^T @ qup; sigmoid(+beta) + 1.
  out    : X *= gate2 broadcast over c (DVE bf16 2x), DMA back.

Program emission is software-pipelined: chunk k's multiply+store is emitted
after chunk k+1's reductions so the DVE stream never stalls on the tiny MLP
latency.  All HBM traffic is bf16 (host converts f32 <-> bf16).
"""

import sys

if "/opt/trn_rl_repo" not in sys.path:
    sys.path.insert(0, "/opt/trn_rl_repo")

import numpy as np

HEAD = 16
B, C, H, W = 256, 2048, 8, 8
NCORES = 8
BPC = B // NCORES          # 32 batches per core
CHUNK_B = 8                # batches per SBUF chunk (8*16 heads = 128 partitions)
NCHUNK = BPC // CHUNK_B    # 4
C16 = C // HEAD            # 128 channels per head group
SPAT = H * W               # 64
FREE = C16 * SPAT          # 8192 elems per partition per chunk

LAST_EXEC_NS = None        # filled when trace=True


def _upsample_matrix():
    """U[8,4]: bilinear 4->8, half-pixel centers (align_corners=False)."""
    U = np.zeros((8, 4), dtype=np.float64)
    for y in range(8):
        src = (y + 0.5) / 2.0 - 0.5
        i0 = int(np.floor(src))
        t = src - i0
        U[y, min(max(i0, 0), 3)] += 1.0 - t
        U[y, min(max(i0 + 1, 0), 3)] += t
    return U


def _pack_params(w1, b1, w2, b2, wv, bv):
    import ml_dtypes

    w1 = np.asarray(w1, np.float64).reshape(HEAD)
    w2 = np.asarray(w2, np.float64).reshape(HEAD)
    b2 = np.asarray(b2, np.float64).reshape(HEAD)
    wv = np.asarray(wv, np.float64).reshape(2)
    bv = float(np.asarray(bv, np.float64))
    b1 = float(np.asarray(b1, np.float64))

    p = np.arange(128)
    h16 = p % HEAD
    blk = p // HEAD  # which batch-slot this partition belongs to

    # fp32 consts [128, 35]: col 0 betacol, col 1 b1, cols 2:34 identity
    # (rows 0:32) for the PE transpose.
    cf32 = np.zeros((128, 35), np.float64)
    cf32[:, 0] = (wv[0] + wv[1]) * b2[h16] + bv
    cf32[:, 1] = b1
    cf32[0:32, 2:34] = np.eye(32)

    U = _upsample_matrix()
    # K2[g, s] = U[y,i] * U[x,j], g = i*4+j, s = y*8+x
    K2 = np.einsum("yi,xj->ijyx", U, U).reshape(16, 64)

    # bf16 consts [128, 208]:
    #   rows 0:32,  cols 0:64    K2E: wv1*K2 for the max half (hcat rows
    #                            0:16), wv0*K2 for the mean half (16:32)
    #   rows 0:128, cols 64:72   W1m[p, b] = (p//16==b) * w1[p%16] / 512
    #   rows 0:128, cols 72:80   W1x[p, b] = (p//16==b) * w1[p%16]
    #   rows 0:8,   cols 80:208  W2blk[b, p] = (p//16==b) * w2[p%16]
    cbf = np.zeros((128, 208), np.float64)
    cbf[0:16, 0:64] = wv[1] * K2
    cbf[16:32, 0:64] = wv[0] * K2
    for b in range(CHUNK_B):
        cbf[:, 64 + b] = np.where(blk == b, w1[h16] / 512.0, 0.0)
        cbf[:, 72 + b] = np.where(blk == b, w1[h16], 0.0)
        cbf[b, 80:208] = np.where(blk == b, w2[h16], 0.0)

    return {
        "cf32": np.ascontiguousarray(cf32, np.float32),
        "cbf": np.ascontiguousarray(cbf.astype(np.float32), ml_dtypes.bfloat16),
    }


def _chain_input_dmas(nc, mybir, window=6):
    """Sliding-window chaining of the x-input quarter-DMAs: quarter i waits
    for quarter i-window.  Unchained, all 16 quarters flood the queues
    round-robin and chunk 0 — which gates all compute — arrives ~4x late;
    fully serialized, too few queues are active to reach full HBM bandwidth
    (one DMA queue sustains only ~100 GB/s).  A window of ~6 keeps ~3 MB in
    flight: saturated HBM and in-order arrival."""
    cum = {}
    xdmas = []
    for fn in nc.m.functions:
        for bb in fn.blocks:
            for ins in bb.instructions:
                si = getattr(ins, "sync_info", None)
                if si is None:
                    continue
                ups = list(si.on_update) if si.on_update else []
                for u in ups:
                    cum[u.id] = cum.get(u.id, 0) + u.update_value
                if (type(ins).__name__ == "InstDMACopy"
                        and str(getattr(ins.ins[0], "memref", "")) == "x"):
                    u = ups[0]
                    xdmas.append((ins, (u.id, u.ant_name, cum[u.id]),
                                  ins.ins[0].offset))
    xdmas.sort(key=lambda t: t[2])
    for i in range(4, len(xdmas)):
        win = 4 if i < 8 else window
        if i < win:
            continue
        sem_id, name, val = xdmas[i - win][1]
        ins = xdmas[i][0]
        w = mybir.SyncWait(sync_type="semaphore", id=sem_id,
                          wait_mode="sem-ge-imm", wait_value=val,
                          ant_name=name)
        ins.sync_info.on_wait = list(ins.sync_info.on_wait or []) + [w]


def _split_multi_waits(nc, mybir):
    """Walrus codegen supports one sync-wait per instruction; hoist extras
    onto standalone InstEventSemaphore waits inserted right before, on the
    same engine (engines execute their stream in order, so this preserves
    the happens-before edges)."""
    n = 0
    for fn in nc.m.functions:
        for bb in fn.blocks:
            out = []
            for ins in bb.instructions:
                si = getattr(ins, "sync_info", None)
                waits = list(si.on_wait) if (si and si.on_wait) else []
                if len(waits) > 1:
                    for w in waits[:-1]:
                        n += 1
                        ev = mybir.InstEventSemaphore(
                            name=f"WSPLIT-{n}",
                            sync_info=mybir.SyncInfo(on_wait=[w], on_update=[]),
                        )
                        ev.engine = ins.engine
                        out.append(ev)
                    si.on_wait = [waits[-1]]
                out.append(ins)
            bb.instructions[:] = out


def _build(split_waits=True):
    import concourse.bass as bass
    import concourse.tile as tile
    from concourse import mybir

    f32 = mybir.dt.float32
    bf16 = mybir.dt.bfloat16
    nc = bass.Bass()

    xd = nc.dram_tensor("x", [NCHUNK, 128, FREE], bf16, kind="ExternalInput")
    od = nc.dram_tensor("out", [NCHUNK, 128, FREE], bf16, kind="ExternalOutput")
    cf32d = nc.dram_tensor("cf32", [128, 35], f32, kind="ExternalInput")
    cbfd = nc.dram_tensor("cbf", [128, 208], bf16, kind="ExternalInput")

    AF = mybir.ActivationFunctionType

    with tile.TileContext(nc) as tc:
        with (
            tc.tile_pool(name="singles", bufs=1) as singles,
            tc.tile_pool(name="xin", bufs=4) as xpool,
            tc.tile_pool(name="mid", bufs=2) as mid,
            tc.tile_pool(name="small", bufs=3) as small,
            tc.tile_pool(name="pmean", bufs=2, space="PSUM") as pmean,
            tc.tile_pool(name="pgpre", bufs=2, space="PSUM") as pgpre,
            tc.tile_pool(name="psmall", bufs=1, space="PSUM") as psmall,
        ):
            # Tiny const DMAs first (the scheduler plans around their
            # completion; issuing them after the bulk input DMAs makes it
            # believe the weights arrive late and it defers the whole PE
            # pipeline).  Then the input DMAs for every chunk: 4 quarter
            # DMAs per chunk on the SP sequencer; the post-pass chains them
            # with a sliding window.
            s_cf32 = singles.tile([128, 35], f32)
            nc.sync.dma_start(out=s_cf32, in_=cf32d[:, :])
            s_cbf = singles.tile([128, 208], bf16)
            nc.sync.dma_start(out=s_cbf, in_=cbfd[:, :])

            # 8 eighth-DMAs per chunk: ~8 concurrent transfers keep enough
            # descriptors in flight to hide HBM latency on all 16 DMA
            # engines.  Chunks 0-1 issue from the (otherwise idle early on)
            # Activation sequencer, which starts ~4us before SP gets through
            # its preamble; chunks 2-3 from SP.
            # tile_wait_until paces the quarters in the SCHEDULER'S sim to
            # match their real (chained) arrival order — without it the sim
            # assumes all 16 quarters flood round-robin, concludes late
            # chunks' data arrives very late, and statically orders their
            # reductions after older chunks' multiplies, which in reality
            # strands the critical path.
            QRT = FREE // 4
            xtiles = []
            for ci in range(NCHUNK):
                X = xpool.tile([128, FREE], bf16, tag="X")
                for q in range(4):
                    nc.sync.dma_start(out=X[:, q * QRT:(q + 1) * QRT],
                                      in_=xd[ci, :, q * QRT:(q + 1) * QRT])
                xtiles.append(X)
            s_beta = s_cf32[:, 0:1]
            s_b1 = s_cf32[:, 1:2]
            s_ident = s_cf32[0:32, 2:34]
            s_k2e = s_cbf[0:32, 0:64]
            s_w1m = s_cbf[:, 64:72]
            s_w1x = s_cbf[:, 72:80]
            s_w2blk = s_cbf[0:8, 80:208]

            # Absorb the const DMAs' semaphores into engine vector clocks
            # (walrus supports one sync-wait per instruction) and pre-load
            # the sigmoid ACT table set during the input-DMA ramp.
            d_a = singles.tile([1, 1], f32, tag="d_a")
            nc.scalar.activation(d_a, s_cf32[0:1, 0:1], AF.Sigmoid)
            d_v = singles.tile([1, 2], bf16, tag="d_v")
            nc.vector.tensor_copy(d_v, s_cbf[0:1, 0:2])
            # GpSimd warmup: absorb the one-time Q7 dispatch/IRAM cost before
            # the multiply offload needs it.
            d_g = singles.tile([128, 2], bf16, tag="d_g")
            nc.gpsimd.tensor_mul(d_g, s_cbf[:, 0:2], s_cbf[:, 0:2])

            # [32,32] staging tile for the mean cells: DVE writes the cell
            # sums into cols 16:32 rows 0:8; a PE transpose then lands them
            # partition-aligned at rows 16:32.  memset once so the PE
            # transpose never reads uninitialized SBUF.
            mean8pad = singles.tile([32, 32], f32, tag="mean8pad")
            nc.vector.memset(mean8pad[:, :], 0.0)

            def emit_reduce(ci):
                """mean (PE) + max (DVE) + MLP -> gate2 for chunk ci.
                Returns (X, gate2) for the deferred multiply/store."""
                X = xtiles[ci]
                Xf = X[:, :]

                # Mean path on TensorE: 16 matmuls over fully CONTIGUOUS
                # 512-column slices of X (strided rhs APs stall the PE
                # stream ~4x).  Since the mean sums over all channels, every
                # slice accumulates into the same PSUM columns (c%8, y, x):
                # MeanP[b, (c8,y,x)] = sum_p W1m[p,b] * sum_slices X.
                meanp = pmean.tile([8, 512], f32, tag="meanp")
                for k in range(16):
                    nc.tensor.matmul(meanp[:, :], s_w1m,
                                     Xf[:, k * 512:(k + 1) * 512],
                                     start=(k == 0), stop=(k == 15))

                # Max path: y-pair fold (two c-halves so each starts as its
                # input quarters land), then a contiguous halving tree, and
                # one strided reduce for the (c2, dx) tail.
                X5 = Xf.rearrange("p (c i dy x) -> p c i dy x",
                                  c=C16, i=4, dy=2, x=8)
                tm = mid.tile([128, 4096], bf16, tag="tm")
                tmv = tm[:, :].rearrange("p (c i x) -> p c i x", c=C16, i=4, x=8)
                # chunk 0's input quarters arrive staggered, so fold per
                # quarter there; later chunks' data is resident by the time
                # the DVE reaches them, so fewer (cheaper) ops win.
                nfold = 4 if ci == 0 else 2
                for ch in range(nfold):
                    cl = slice(ch * (C16 // nfold), (ch + 1) * (C16 // nfold))
                    nc.vector.tensor_max(
                        tmv[:, cl], X5[:, cl, :, 0, :], X5[:, cl, :, 1, :])
                n = 4096
                while n > 64:
                    n //= 2
                    nc.vector.tensor_max(tm[:, 0:n], tm[:, 0:n], tm[:, n:2 * n])
                smax = small.tile([128, 16], bf16, tag="smax")
                tmd = tm[:, 0:64].rearrange("p (c i j dx) -> p i j c dx",
                                            c=2, i=4, j=4, dx=2)
                nc.vector.reduce_max(out=smax[:, :], in_=tmd,
                                     axis=mybir.AxisListType.XY)

                # mean cells: ScalarE copies the PSUM partials to SBUF (DVE
                # can read only one PSUM operand per op), DVE folds the c8
                # halves at 2x and reduces the 2x2 patch, then a PE
                # transpose lands the cells on partitions 16:32.
                meanpS = small.tile([8, 512], bf16, tag="meanpS")
                nc.scalar.copy(meanpS[:, :], meanp[:, :])
                for n in (256, 128, 64):
                    nc.vector.tensor_add(meanpS[:, 0:n], meanpS[:, 0:n],
                                         meanpS[:, n:2 * n])
                mpv = meanpS[:, 0:64].rearrange("p (i dy j dx) -> p i j dy dx",
                                                i=4, dy=2, j=4, dx=2)
                nc.vector.reduce_sum(out=mean8pad[0:8, 16:32], in_=mpv,
                                     axis=mybir.AxisListType.XY)
                meant = psmall.tile([32, 32], f32, tag="meant")
                nc.tensor.transpose(meant[:, :], mean8pad[:, :], s_ident)
                # hpre_max[cell, b] overwrites rows 0:16 of the same PSUM
                # tile (PSUM reads must start at partition 0, so both halves
                # must live in one offset-0 tile for a single relu).
                nc.tensor.matmul(meant[0:16, 0:8], smax[:, :], s_w1x)

                # hcatT [32, 8]: rows 0:16 = relu(hpre_max + b1),
                # rows 16:32 = relu(hpre_mean + b1)
                hcat = small.tile([32, 8], bf16, tag="hcat")
                nc.scalar.activation(hcat[:, :], meant[0:32, 0:8], AF.Relu,
                                     bias=s_b1[0:32, :])

                qup = psmall.tile([8, 64], f32, tag="qup")
                nc.tensor.matmul(qup[:, :], hcat[:, :], s_k2e)
                qupS = small.tile([8, 64], bf16, tag="qupS")
                nc.scalar.copy(qupS[:, :], qup[:, :])

                gpre = pgpre.tile([128, 64], f32, tag="gpre")
                nc.tensor.matmul(gpre[:, :], s_w2blk, qupS[:, :])

                gate = small.tile([128, 64], bf16, tag="gate")
                nc.scalar.activation(gate[:, :], gpre[:, :], AF.Sigmoid,
                                     bias=s_beta)
                gate2 = small.tile([128, 64], bf16, tag="gate2")
                nc.scalar.add(gate2[:, :], gate[:, :], 1.0)
                return X, gate2

            def emit_mul_store(ci, X, gate2):
                """out = gate2 * x (gate2 broadcast over the 128 group
                channels), store.  Pieces let the store overlap the mul.
                Output DMAs issue on the SP sequencer: by the time the first
                store is ready all chained input DMAs have issued, while the
                Activation sequencer must stay free for the MLP chain (a
                pending out-DMA's wait would stall the next chunk's
                relu/sigmoid and delay its gate).  The last chunk's stores
                are split into two parallel streams each so the final drain
                is not single-stream limited."""
                nstream = 2 if ci == NCHUNK - 1 else 1
                for cl in ((0, 32), (32, 64), (64, 96), (96, 128)):
                    cs = cl[1] - cl[0]
                    sl = slice(cl[0] * SPAT, cl[1] * SPAT)
                    g_bc = gate2[:, :].unsqueeze(1).broadcast_to([128, cs, SPAT])
                    X3 = X[:, sl].rearrange("p (c s) -> p c s", s=SPAT)
                    nc.vector.tensor_mul(X3, g_bc, X3)
                    step = cs * SPAT // nstream
                    for st in range(nstream):
                        ssl = slice(sl.start + st * step,
                                    sl.start + (st + 1) * step)
                        nc.sync.dma_start(out=od[ci, :, ssl], in_=X[:, ssl])

            # Defer each chunk's multiply by TWO chunks: every gate is then
            # ~6us-deep ready when its multiply's turn comes, so neither the
            # kernel tail nor scheduler reorderings ever wait on the tiny
            # MLP chain's latency.
            LAG = 2
            pend = []
            for ci in range(NCHUNK):
                pend.append(emit_reduce(ci))
                if ci >= LAG:
                    emit_mul_store(ci - LAG, *pend[ci - LAG])
            for ci in range(NCHUNK - LAG, NCHUNK):
                emit_mul_store(ci, *pend[ci])

    _chain_input_dmas(nc, mybir, window=6)
    if split_waits:
        _split_multi_waits(nc, mybir)
    return nc


def _shard_inputs(x, consts):
    import ml_dtypes

    xb = np.ascontiguousarray(x).astype(ml_dtypes.bfloat16)
    in_maps = []
    for i in range(NCORES):
        shard = xb[i * BPC:(i + 1) * BPC]  # [32, 2048, 8, 8]
        m = {"x": np.ascontiguousarray(shard.reshape(NCHUNK, 128, FREE))}
        m.update(consts)
        in_maps.append(m)
    return in_maps


def kernel(x, w1, b1, w2, b2, wv, bv, trace=False):
    global LAST_EXEC_NS
    from concourse.bass_utils import run_bass_kernel_spmd

    x = np.asarray(x, np.float32)
    consts = _pack_params(w1, b1, w2, b2, wv, bv)
    nc = _build()
    in_maps = _shard_inputs(x, consts)

    res = run_bass_kernel_spmd(nc, in_maps, core_ids=list(range(NCORES)),
                               trace=trace)
    LAST_EXEC_NS = res.exec_time_ns

    out = np.empty((B, C, H, W), np.float32)
    for i, r in enumerate(res.results):
        out[i * BPC:(i + 1) * BPC] = np.asarray(r["out"], np.float32).reshape(
            BPC, C, H, W)
    return out
